# revision 13
# baseline (speedup 1.0000x reference)
"""Trainium2 Bass kernel for nn_Conv2D3_72026601554290.

Reference computation:
    h = conv7x7_valid(x[4,3,70,70], W1[64,3,7,7]) + b1      -> [4,64,64,64]
    repeat 200x: h = W2 @ h + b2   (1x1 conv, shared weights)

Strategy:
  * The 200 repeated affine steps share one weight matrix, so the tail of
    the network is the affine map h -> W2^200 h + (sum_k W2^k) b2.  We fold
    W2^FUSE (computed in float64 on the host, rounded to f32) into the
    device program: the device runs 200/FUSE GEMM steps.  FUSE=200 folds
    everything into the conv weights (a single fused conv).  Numerics vs
    the f32 reference are ~1e-6 for every FUSE (spectral radius of W2 is
    0.979; all intermediate values stay tiny).
  * Data parallel across 8 NeuronCores: 16384 output positions -> 2048 per
    core (half an image each).  No cross-device communication.
  * Conv is an im2col GEMM with the bias folded in as a constant-1 row:
    K = 3*7*7 + 1 = 148, split into accumulating K=128 + K=20 matmuls.
  * Matmul dtype modes:
      - "f32r"  : TF32 operands (pre-rounded on host), 1 cycle/row.
      - "f32r3" : each operand split hi+lo TF32 terms; 3 matmuls
                  (hi*hi + hi*lo + lo*hi) -> fp32-grade accuracy at 3/8
                  the cost of true fp32 matmul.
      - "f32"   : plain fp32 (2 HW passes, 8 cycles/row).
    f32r matmuls require dst PSUM base partition 0; outputs land in four
    [64, 512] PSUM tiles and the Vector/Scalar engines copy them (with a
    +64 partition shift for the second position group) into the [128,
    1024] output layout.
"""

import numpy as np

import concourse.bacc as bacc
import concourse.tile as tile
import concourse.mybir as mybir
from concourse.bass_utils import run_bass_kernel_spmd

F32 = mybir.dt.float32
F32R = mybir.dt.float32r

N_CORES = 8
N_REPEAT = 200
FUSE = 200  # device runs N_REPEAT//FUSE GEMM steps; 200 -> fully folded conv
MM_DTYPE = "f32r"  # "f32" | "f32r" | "f32r3"
WARMUP_MMS = 0  # f32r matmuls run at fixed rate; PE warmup does not help
POS_PER_CORE = 2048  # 4*64*64 / 8
HALF = POS_PER_CORE // 2  # free-dim size of the [128, 1024] output layout
OH = OW = 64
KH = KW = 7
CIN = 3
CH = 64
K_IM = CIN * KH * KW + 1  # 148: im2col rows + constant-1 bias row
K_LO = K_IM - 128  # 20

_cache = {}


def _build_nc(steps, mode):
    """Build + compile the per-core Bass program (same NEFF for all cores)."""
    nterm = {"f32": 1, "f32r": 1, "f32r3": 2}[mode]  # operand split terms
    mdt = F32 if mode == "f32" else F32R
    warmup = WARMUP_MMS if steps == 0 else 0
    nc = bacc.Bacc("TRN2", target_bir_lowering=False, debug=False,
                   num_devices=N_CORES)

    # conv weights: [K_IM, 64] lhsT layout, one tensor per split term
    wc_ext = [nc.declare_dram_parameter(f"wc{t}", [K_IM, CH], mdt, isOutput=False)
              for t in range(nterm)]
    im_ext = [nc.declare_dram_parameter(f"im{t}", [K_IM, POS_PER_CORE], mdt,
                                        isOutput=False)
              for t in range(nterm)]
    if steps:
        wl_ext = nc.declare_dram_parameter("wl", [128, CH + 1], F32, isOutput=False)
    o_ext = nc.declare_dram_parameter("o", [128, HALF], F32, isOutput=True)

    with tile.TileContext(nc) as tc:
        with (
            tc.tile_pool(name="const", bufs=1) as cpool,
            tc.tile_pool(name="act", bufs=2) as apool,
            tc.tile_pool(name="psum", bufs=1, space="PSUM") as ppool,
        ):
            wch = [cpool.tile([128, CH], mdt, name=f"wch{t}_sb") for t in range(nterm)]
            wcl = [cpool.tile([K_LO, CH], mdt, name=f"wcl{t}_sb") for t in range(nterm)]
            imh = [[cpool.tile([128, 512], mdt, name=f"imh{t}c{c}_sb")
                    for c in range(4)] for t in range(nterm)]
            iml = [cpool.tile([K_LO, POS_PER_CORE], mdt, name=f"iml{t}_sb")
                   for t in range(nterm)]
            for t in range(nterm):
                nc.sync.dma_start(wch[t][:], wc_ext[t][0:128, :])
                nc.sync.dma_start(wcl[t][:], wc_ext[t][128:K_IM, :])
            if steps:
                wl = cpool.tile([128, CH + 1], F32)
                nc.sync.dma_start(wl[:], wl_ext[:])
            # prime the scalar-engine activation table while DMAs run
            scratch = apool.tile([128, 1], F32, tag="scratch")
            nc.scalar.activation(scratch[:], wch[0][:, 0:1].bitcast(F32),
                                 mybir.ActivationFunctionType.Identity)
            if warmup:
                warm_rhs = cpool.tile([128, 512], mybir.dt.bfloat16, name="warm_rhs")
                nc.vector.memset(warm_rhs[:], 0.0)
            tc.strict_bb_all_engine_barrier()
            # chunked im2col loads into separate per-chunk tiles; each
            # conv chunk's matmuls depend only on its own DMA
            for t in range(nterm):
                for c in range(4):
                    cs = slice(c * 512, (c + 1) * 512)
                    nc.sync.dma_start(imh[t][c][:], im_ext[t][0:128, cs])
                nc.sync.dma_start(iml[t][:], im_ext[t][128:K_IM, :])
            if warmup:
                # warm up the PE clock (HAM) while the im2col DMAs stream
                warm_ps = ppool.tile([64, 512], F32, name="warm_ps")
                for t in range(warmup):
                    nc.tensor.matmul(warm_ps[:], warm_rhs[:, 0:64], warm_rhs[:],
                                     start=True, stop=True, tile_position=(0, 0))

            # ---- conv GEMM: 4 chunks of 512 positions, dst PSUM partitions 0:64
            if nterm == 1:
                pairs = [(0, 0)]
            else:  # hi*hi + hi*lo + lo*hi  (lo*lo term negligible)
                pairs = [(0, 0), (0, 1), (1, 0)]
            ps = [ppool.tile([64, 512], F32, name=f"ps{c}") for c in range(4)]
            for c in range(4):
                cs = slice(c * 512, (c + 1) * 512)
                n = len(pairs)
                for i, (tw, tx) in enumerate(pairs):
                    nc.tensor.matmul(ps[c][:], wch[tw][:], imh[tx][c][:],
                                     start=(i == 0), stop=False,
                                     tile_position=(0, 0))
                    nc.tensor.matmul(ps[c][:], wcl[tw][:], iml[tx][:, cs],
                                     start=False, stop=(i == n - 1),
                                     tile_position=(0, 0))

            # ---- copies into [128, 1024] layout (+64 partition shift for B),
            # each followed by its own output-store DMA
            h = apool.tile([128, HALF], F32, tag="h")
            nc.vector.tensor_copy(h[0:64, 0:512], ps[0][:])
            if steps == 0:
                nc.sync.dma_start(o_ext[0:64, 0:512], h[0:64, 0:512])
            nc.scalar.copy(h[0:64, 512:1024], ps[1][:])
            if steps == 0:
                nc.scalar.dma_start(o_ext[0:64, 512:1024], h[0:64, 512:1024])
            nc.vector.tensor_copy(h[64:128, 0:512], ps[2][:])
            if steps == 0:
                nc.sync.dma_start(o_ext[64:128, 0:512], h[64:128, 0:512])
            nc.scalar.copy(h[64:128, 512:1024], ps[3][:])
            if steps == 0:
                nc.scalar.dma_start(o_ext[64:128, 512:1024], h[64:128, 512:1024])

            # ---- fused GEMM steps (fp32 exact): h <- P_FUSE @ h + c_FUSE
            for s in range(steps):
                bl = wl[:, CH:CH + 1]
                psA = ppool.tile([128, 512], F32, name="psA", bufs=2)
                psB = ppool.tile([128, 512], F32, name="psB", bufs=2)
                nc.tensor.matmul(psA[0:64, :], wl[0:64, 0:CH], h[0:64, 0:512],
                                 start=True, stop=True, tile_position=(0, 0))
                nc.tensor.matmul(psA[64:128, :], wl[0:64, 0:CH], h[0:64, 512:1024],
                                 start=True, stop=True, tile_position=(0, 64))
                nc.tensor.matmul(psB[0:64, :], wl[64:128, 0:CH], h[64:128, 0:512],
                                 start=True, stop=True, tile_position=(64, 0))
                nc.tensor.matmul(psB[64:128, :], wl[64:128, 0:CH],
                                 h[64:128, 512:1024],
                                 start=True, stop=True, tile_position=(64, 64))
                last = s == steps - 1
                h_new = apool.tile([128, HALF], F32, tag="h")
                nc.vector.tensor_scalar(h_new[:, 0:512], psA[:], bl, None,
                                        mybir.AluOpType.add)
                nc.scalar.activation(h_new[:, 512:1024], psB[:],
                                     mybir.ActivationFunctionType.Identity,
                                     bias=bl)
                if last:
                    nc.sync.dma_start(o_ext[:, 0:512], h_new[:, 0:512])
                    nc.sync.dma_start(o_ext[:, 512:1024], h_new[:, 512:1024])
                h = h_new

    nc.compile()
    return nc


def _fold(W1, b1, W2, b2, fuse):
    """Fold `fuse` affine steps into the conv weights (float64 host math).

    Returns (Wc [64,148] incl bias column, Pk [64,64]|None, ck [64]|None).
    """
    W2d = W2.astype(np.float64)
    W1m = W1.reshape(CH, K_IM - 1).astype(np.float64)

    def affine_pow(k):
        # (P, S) with P = W2^k, S = sum_{j<k} W2^j  via binary doubling
        P = np.eye(CH)
        S = np.zeros((CH, CH))
        base_P = W2d
        base_S = np.eye(CH)
        while k:
            if k & 1:
                S = base_S + base_P @ S
                P = base_P @ P
            base_S = base_S + base_P @ base_S
            base_P = base_P @ base_P
            k >>= 1
        return P, S

    if fuse == N_REPEAT:
        P, S = affine_pow(N_REPEAT)
        Wm = P @ W1m
        bias = P @ b1.astype(np.float64) + S @ b2.astype(np.float64)
    else:
        Wm = W1m
        bias = b1.astype(np.float64)
    Wc = np.concatenate([Wm, bias[:, None]], axis=1)  # [64, 148]
    if fuse == N_REPEAT:
        return Wc, None, None
    P, S = affine_pow(fuse)
    return Wc, P.astype(np.float32), (S @ b2.astype(np.float64)).astype(np.float32)


def _im2col_core(x, core):
    """im2col + constant-1 bias row for this core -> [148, 2048] f64->f32."""
    b = core // 2
    y0 = 32 * (core % 2)
    cols = np.empty((K_IM, POS_PER_CORE), np.float32)
    i = 0
    for c in range(CIN):
        for dy in range(KH):
            for dx in range(KW):
                cols[i] = x[b, c, y0 + dy:y0 + dy + 32, dx:dx + OW].reshape(-1)
                i += 1
    cols[i] = 1.0
    return cols


def _tf32_round(a):
    """Round f32 array to tf32 (10-bit mantissa), round-to-nearest-even."""
    a = np.ascontiguousarray(a, dtype=np.float32)
    u = a.view(np.uint32)
    lsb = (u >> 13) & 1
    out = ((u + 0x0FFF + lsb) & 0xFFFFE000).astype(np.uint32)
    return out.view(np.float32)


def _split_terms(a, mode):
    """Operand splitting per matmul dtype mode -> list of arrays."""
    if mode == "f32":
        return [np.ascontiguousarray(a, dtype=np.float32)]
    hi = _tf32_round(a)
    if mode == "f32r":
        return [hi]
    lo = _tf32_round(np.asarray(a, np.float32) - hi)
    return [hi, lo]


def _run(x, W1, b1, W2, b2, trace=False):
    x = np.asarray(x, dtype=np.float32)
    W1 = np.asarray(W1, dtype=np.float32)
    b1 = np.asarray(b1, dtype=np.float32)
    W2 = np.asarray(W2, dtype=np.float32)
    b2 = np.asarray(b2, dtype=np.float32)

    steps = 0 if FUSE == N_REPEAT else N_REPEAT // FUSE
    if steps:
        assert steps * FUSE == N_REPEAT

    key = (steps, MM_DTYPE, WARMUP_MMS)
    if _cache.get("key") != key:
        _cache["nc"] = _build_nc(steps, MM_DTYPE)
        _cache["key"] = key
    nc = _cache["nc"]

    nterm = {"f32": 1, "f32r": 1, "f32r3": 2}[MM_DTYPE]  # operand terms

    Wc, Pk, ck = _fold(W1, b1, W2, b2, FUSE)
    WcT = np.ascontiguousarray(Wc.T)  # [148, 64] lhsT layout
    w_terms = _split_terms(WcT, MM_DTYPE)

    shared = {f"wc{t}": w_terms[t] for t in range(len(w_terms))}
    if steps:
        PkT = np.ascontiguousarray(Pk.T)
        wl = np.concatenate([PkT, PkT], axis=0).astype(np.float32)
        bl = np.concatenate([ck, ck])[:, None].astype(np.float32)
        shared["wl"] = np.concatenate([wl, bl], axis=1)

    in_maps = []
    for core in range(N_CORES):
        cols = _im2col_core(x, core)
        x_terms = _split_terms(cols, MM_DTYPE)
        m = dict(shared)
        for t, arr in enumerate(x_terms):
            m[f"im{t}"] = arr
        in_maps.append(m)

    res = run_bass_kernel_spmd(nc, in_maps, list(range(N_CORES)), trace=trace)

    out = np.empty((4, CH, OH, OW), np.float32)
    for core in range(N_CORES):
        o = res.results[core]["o"].copy()
        if steps % 2 == 1:
            # undo the per-step quarter-block swap (Q2 <-> Q3)
            tmp = o[0:64, 512:1024].copy()
            o[0:64, 512:1024] = o[64:128, 0:512]
            o[64:128, 0:512] = tmp
        b = core // 2
        y0 = 32 * (core % 2)
        # group A = local positions 0..1023 (16 rows), group B = 1024..2047
        out[b, :, y0:y0 + 16, :] = o[0:64].reshape(CH, 16, OW)
        out[b, :, y0 + 16:y0 + 32, :] = o[64:128].reshape(CH, 16, OW)
    return out, res


def kernel(**inputs):
    out, _ = _run(inputs["x"], inputs["W1"], inputs["b1"],
                  inputs["W2"], inputs["b2"], trace=False)
    return out


def kernel_traced(**inputs):
    """Like kernel() but with NTFF hardware profiling; returns (out, res)."""
    import sys
    import types
    if "antenv.axon_hooks" not in sys.modules:
        from trn_agent_boot.trn_boot import _ntff_profile_via_ctypes
        hook = _ntff_profile_via_ctypes("/opt/axon/libaxon_pjrt.so")
        mod = types.ModuleType("antenv.axon_hooks")
        mod.get_axon_ntff_profile_hook = lambda: hook
        mod.set_axon_ntff_profile_hook = lambda h: None
        sys.modules["antenv.axon_hooks"] = mod
    return _run(inputs["x"], inputs["W1"], inputs["b1"],
                inputs["W2"], inputs["b2"], trace=True)


# revision 14
# speedup vs baseline: 1.0315x; 1.0315x over previous
"""Trainium2 Bass kernel for nn_Conv2D3_72026601554290.

Reference computation:
    h = conv7x7_valid(x[4,3,70,70], W1[64,3,7,7]) + b1      -> [4,64,64,64]
    repeat 200x: h = W2 @ h + b2   (1x1 conv, shared weights)

Strategy:
  * The 200 repeated affine steps share one weight matrix, so the tail of
    the network is the affine map h -> W2^200 h + (sum_k W2^k) b2.  We fold
    W2^FUSE (computed in float64 on the host, rounded to f32) into the
    device program: the device runs 200/FUSE GEMM steps.  FUSE=200 folds
    everything into the conv weights (a single fused conv).  Numerics vs
    the f32 reference are ~1e-6 for every FUSE (spectral radius of W2 is
    0.979; all intermediate values stay tiny).
  * Data parallel across 8 NeuronCores: 16384 output positions -> 2048 per
    core (half an image each).  No cross-device communication.
  * Conv is an im2col GEMM with the bias folded in as a constant-1 row:
    K = 3*7*7 + 1 = 148, split into accumulating K=128 + K=20 matmuls.
  * Matmul dtype modes:
      - "f32r"  : TF32 operands (pre-rounded on host), 1 cycle/row.
      - "f32r3" : each operand split hi+lo TF32 terms; 3 matmuls
                  (hi*hi + hi*lo + lo*hi) -> fp32-grade accuracy at 3/8
                  the cost of true fp32 matmul.
      - "f32"   : plain fp32 (2 HW passes, 8 cycles/row).
    f32r matmuls require dst PSUM base partition 0; outputs land in four
    [64, 512] PSUM tiles and the Vector/Scalar engines copy them (with a
    +64 partition shift for the second position group) into the [128,
    1024] output layout.
"""

import numpy as np

import concourse.bacc as bacc
import concourse.tile as tile
import concourse.mybir as mybir
from concourse.bass_utils import run_bass_kernel_spmd

F32 = mybir.dt.float32
F32R = mybir.dt.float32r

N_CORES = 8
N_REPEAT = 200
FUSE = 200  # device runs N_REPEAT//FUSE GEMM steps; 200 -> fully folded conv
MM_DTYPE = "f32r"  # "f32" | "f32r" | "f32r3"
WARMUP_MMS = 0  # f32r matmuls run at fixed rate; PE warmup does not help
POS_PER_CORE = 2048  # 4*64*64 / 8
HALF = POS_PER_CORE // 2  # free-dim size of the [128, 1024] output layout
OH = OW = 64
KH = KW = 7
CIN = 3
CH = 64
K_IM = CIN * KH * KW + 1  # 148: im2col rows + constant-1 bias row
K_LO = K_IM - 128  # 20

_cache = {}


def _build_nc(steps, mode):
    """Build + compile the per-core Bass program (same NEFF for all cores)."""
    nterm = {"f32": 1, "f32r": 1, "f32r3": 2}[mode]  # operand split terms
    mdt = F32 if mode == "f32" else F32R
    warmup = WARMUP_MMS if steps == 0 else 0
    nc = bacc.Bacc("TRN2", target_bir_lowering=False, debug=False,
                   num_devices=N_CORES)

    # conv weights: [K_IM, 64] lhsT layout, one tensor per split term
    wc_ext = [nc.declare_dram_parameter(f"wc{t}", [K_IM, CH], mdt, isOutput=False)
              for t in range(nterm)]
    im_ext = [nc.declare_dram_parameter(f"im{t}", [K_IM, POS_PER_CORE], mdt,
                                        isOutput=False)
              for t in range(nterm)]
    if steps:
        wl_ext = nc.declare_dram_parameter("wl", [128, CH + 1], F32, isOutput=False)
    o_ext = nc.declare_dram_parameter("o", [128, HALF], F32, isOutput=True)

    with tile.TileContext(nc) as tc:
        with (
            tc.tile_pool(name="const", bufs=1) as cpool,
            tc.tile_pool(name="act", bufs=2) as apool,
            tc.tile_pool(name="psum", bufs=1, space="PSUM") as ppool,
        ):
            wch = [cpool.tile([128, CH], mdt, name=f"wch{t}_sb") for t in range(nterm)]
            wcl = [cpool.tile([K_LO, CH], mdt, name=f"wcl{t}_sb") for t in range(nterm)]
            imh = [[cpool.tile([128, 512], mdt, name=f"imh{t}c{c}_sb")
                    for c in range(4)] for t in range(nterm)]
            iml = [cpool.tile([K_LO, POS_PER_CORE], mdt, name=f"iml{t}_sb")
                   for t in range(nterm)]
            for t in range(nterm):
                nc.sync.dma_start(wch[t][:], wc_ext[t][0:128, :])
                nc.sync.dma_start(wcl[t][:], wc_ext[t][128:K_IM, :])
            if steps:
                wl = cpool.tile([128, CH + 1], F32)
                nc.sync.dma_start(wl[:], wl_ext[:])
            if warmup:
                warm_rhs = cpool.tile([128, 512], mybir.dt.bfloat16, name="warm_rhs")
                nc.vector.memset(warm_rhs[:], 0.0)
            tc.strict_bb_all_engine_barrier()
            # prime the scalar-engine activation table while DMAs stream
            # (after the barrier so it cannot delay the im2col triggers)
            scratch = apool.tile([128, 1], F32, tag="scratch")
            nc.scalar.activation(scratch[:], wch[0][:, 0:1].bitcast(F32),
                                 mybir.ActivationFunctionType.Identity)
            # chunked im2col loads into separate per-chunk tiles; the small
            # K=20 tail goes first so each chunk's accumulation can close
            # (and its copy start) as soon as that chunk's main DMA lands
            for t in range(nterm):
                nc.sync.dma_start(iml[t][:], im_ext[t][128:K_IM, :])
                for c in range(4):
                    cs = slice(c * 512, (c + 1) * 512)
                    nc.sync.dma_start(imh[t][c][:], im_ext[t][0:128, cs])
            if warmup:
                # warm up the PE clock (HAM) while the im2col DMAs stream
                warm_ps = ppool.tile([64, 512], F32, name="warm_ps")
                for t in range(warmup):
                    nc.tensor.matmul(warm_ps[:], warm_rhs[:, 0:64], warm_rhs[:],
                                     start=True, stop=True, tile_position=(0, 0))

            # ---- conv GEMM: 4 chunks of 512 positions, dst PSUM partitions 0:64
            if nterm == 1:
                pairs = [(0, 0)]
            else:  # hi*hi + hi*lo + lo*hi  (lo*lo term negligible)
                pairs = [(0, 0), (0, 1), (1, 0)]
            ps = [ppool.tile([64, 512], F32, name=f"ps{c}") for c in range(4)]
            for c in range(4):
                cs = slice(c * 512, (c + 1) * 512)
                n = len(pairs)
                for i, (tw, tx) in enumerate(pairs):
                    nc.tensor.matmul(ps[c][:], wch[tw][:], imh[tx][c][:],
                                     start=(i == 0), stop=False,
                                     tile_position=(0, 0))
                    nc.tensor.matmul(ps[c][:], wcl[tw][:], iml[tx][:, cs],
                                     start=False, stop=(i == n - 1),
                                     tile_position=(0, 0))

            # ---- copies into [128, 1024] layout (+64 partition shift for B),
            # each followed by its own output-store DMA
            h = apool.tile([128, HALF], F32, tag="h")
            nc.vector.tensor_copy(h[0:64, 0:512], ps[0][:])
            if steps == 0:
                nc.sync.dma_start(o_ext[0:64, 0:512], h[0:64, 0:512])
            nc.scalar.copy(h[0:64, 512:1024], ps[1][:])
            if steps == 0:
                nc.scalar.dma_start(o_ext[0:64, 512:1024], h[0:64, 512:1024])
            nc.vector.tensor_copy(h[64:128, 0:512], ps[2][:])
            if steps == 0:
                nc.sync.dma_start(o_ext[64:128, 0:512], h[64:128, 0:512])
            nc.scalar.copy(h[64:128, 512:1024], ps[3][:])
            if steps == 0:
                nc.scalar.dma_start(o_ext[64:128, 512:1024], h[64:128, 512:1024])

            # ---- fused GEMM steps (fp32 exact): h <- P_FUSE @ h + c_FUSE
            for s in range(steps):
                bl = wl[:, CH:CH + 1]
                psA = ppool.tile([128, 512], F32, name="psA", bufs=2)
                psB = ppool.tile([128, 512], F32, name="psB", bufs=2)
                nc.tensor.matmul(psA[0:64, :], wl[0:64, 0:CH], h[0:64, 0:512],
                                 start=True, stop=True, tile_position=(0, 0))
                nc.tensor.matmul(psA[64:128, :], wl[0:64, 0:CH], h[0:64, 512:1024],
                                 start=True, stop=True, tile_position=(0, 64))
                nc.tensor.matmul(psB[0:64, :], wl[64:128, 0:CH], h[64:128, 0:512],
                                 start=True, stop=True, tile_position=(64, 0))
                nc.tensor.matmul(psB[64:128, :], wl[64:128, 0:CH],
                                 h[64:128, 512:1024],
                                 start=True, stop=True, tile_position=(64, 64))
                last = s == steps - 1
                h_new = apool.tile([128, HALF], F32, tag="h")
                nc.vector.tensor_scalar(h_new[:, 0:512], psA[:], bl, None,
                                        mybir.AluOpType.add)
                nc.scalar.activation(h_new[:, 512:1024], psB[:],
                                     mybir.ActivationFunctionType.Identity,
                                     bias=bl)
                if last:
                    nc.sync.dma_start(o_ext[:, 0:512], h_new[:, 0:512])
                    nc.sync.dma_start(o_ext[:, 512:1024], h_new[:, 512:1024])
                h = h_new

    nc.compile()
    return nc


def _fold(W1, b1, W2, b2, fuse):
    """Fold `fuse` affine steps into the conv weights (float64 host math).

    Returns (Wc [64,148] incl bias column, Pk [64,64]|None, ck [64]|None).
    """
    W2d = W2.astype(np.float64)
    W1m = W1.reshape(CH, K_IM - 1).astype(np.float64)

    def affine_pow(k):
        # (P, S) with P = W2^k, S = sum_{j<k} W2^j  via binary doubling
        P = np.eye(CH)
        S = np.zeros((CH, CH))
        base_P = W2d
        base_S = np.eye(CH)
        while k:
            if k & 1:
                S = base_S + base_P @ S
                P = base_P @ P
            base_S = base_S + base_P @ base_S
            base_P = base_P @ base_P
            k >>= 1
        return P, S

    if fuse == N_REPEAT:
        P, S = affine_pow(N_REPEAT)
        Wm = P @ W1m
        bias = P @ b1.astype(np.float64) + S @ b2.astype(np.float64)
    else:
        Wm = W1m
        bias = b1.astype(np.float64)
    Wc = np.concatenate([Wm, bias[:, None]], axis=1)  # [64, 148]
    if fuse == N_REPEAT:
        return Wc, None, None
    P, S = affine_pow(fuse)
    return Wc, P.astype(np.float32), (S @ b2.astype(np.float64)).astype(np.float32)


def _im2col_core(x, core):
    """im2col + constant-1 bias row for this core -> [148, 2048] f64->f32."""
    b = core // 2
    y0 = 32 * (core % 2)
    cols = np.empty((K_IM, POS_PER_CORE), np.float32)
    i = 0
    for c in range(CIN):
        for dy in range(KH):
            for dx in range(KW):
                cols[i] = x[b, c, y0 + dy:y0 + dy + 32, dx:dx + OW].reshape(-1)
                i += 1
    cols[i] = 1.0
    return cols


def _tf32_round(a):
    """Round f32 array to tf32 (10-bit mantissa), round-to-nearest-even."""
    a = np.ascontiguousarray(a, dtype=np.float32)
    u = a.view(np.uint32)
    lsb = (u >> 13) & 1
    out = ((u + 0x0FFF + lsb) & 0xFFFFE000).astype(np.uint32)
    return out.view(np.float32)


def _split_terms(a, mode):
    """Operand splitting per matmul dtype mode -> list of arrays."""
    if mode == "f32":
        return [np.ascontiguousarray(a, dtype=np.float32)]
    hi = _tf32_round(a)
    if mode == "f32r":
        return [hi]
    lo = _tf32_round(np.asarray(a, np.float32) - hi)
    return [hi, lo]


def _run(x, W1, b1, W2, b2, trace=False):
    x = np.asarray(x, dtype=np.float32)
    W1 = np.asarray(W1, dtype=np.float32)
    b1 = np.asarray(b1, dtype=np.float32)
    W2 = np.asarray(W2, dtype=np.float32)
    b2 = np.asarray(b2, dtype=np.float32)

    steps = 0 if FUSE == N_REPEAT else N_REPEAT // FUSE
    if steps:
        assert steps * FUSE == N_REPEAT

    key = (steps, MM_DTYPE, WARMUP_MMS)
    if _cache.get("key") != key:
        _cache["nc"] = _build_nc(steps, MM_DTYPE)
        _cache["key"] = key
    nc = _cache["nc"]

    nterm = {"f32": 1, "f32r": 1, "f32r3": 2}[MM_DTYPE]  # operand terms

    Wc, Pk, ck = _fold(W1, b1, W2, b2, FUSE)
    WcT = np.ascontiguousarray(Wc.T)  # [148, 64] lhsT layout
    w_terms = _split_terms(WcT, MM_DTYPE)

    shared = {f"wc{t}": w_terms[t] for t in range(len(w_terms))}
    if steps:
        PkT = np.ascontiguousarray(Pk.T)
        wl = np.concatenate([PkT, PkT], axis=0).astype(np.float32)
        bl = np.concatenate([ck, ck])[:, None].astype(np.float32)
        shared["wl"] = np.concatenate([wl, bl], axis=1)

    in_maps = []
    for core in range(N_CORES):
        cols = _im2col_core(x, core)
        x_terms = _split_terms(cols, MM_DTYPE)
        m = dict(shared)
        for t, arr in enumerate(x_terms):
            m[f"im{t}"] = arr
        in_maps.append(m)

    res = run_bass_kernel_spmd(nc, in_maps, list(range(N_CORES)), trace=trace)

    out = np.empty((4, CH, OH, OW), np.float32)
    for core in range(N_CORES):
        o = res.results[core]["o"].copy()
        if steps % 2 == 1:
            # undo the per-step quarter-block swap (Q2 <-> Q3)
            tmp = o[0:64, 512:1024].copy()
            o[0:64, 512:1024] = o[64:128, 0:512]
            o[64:128, 0:512] = tmp
        b = core // 2
        y0 = 32 * (core % 2)
        # group A = local positions 0..1023 (16 rows), group B = 1024..2047
        out[b, :, y0:y0 + 16, :] = o[0:64].reshape(CH, 16, OW)
        out[b, :, y0 + 16:y0 + 32, :] = o[64:128].reshape(CH, 16, OW)
    return out, res


def kernel(**inputs):
    out, _ = _run(inputs["x"], inputs["W1"], inputs["b1"],
                  inputs["W2"], inputs["b2"], trace=False)
    return out


def kernel_traced(**inputs):
    """Like kernel() but with NTFF hardware profiling; returns (out, res)."""
    import sys
    import types
    if "antenv.axon_hooks" not in sys.modules:
        from trn_agent_boot.trn_boot import _ntff_profile_via_ctypes
        hook = _ntff_profile_via_ctypes("/opt/axon/libaxon_pjrt.so")
        mod = types.ModuleType("antenv.axon_hooks")
        mod.get_axon_ntff_profile_hook = lambda: hook
        mod.set_axon_ntff_profile_hook = lambda h: None
        sys.modules["antenv.axon_hooks"] = mod
    return _run(inputs["x"], inputs["W1"], inputs["b1"],
                inputs["W2"], inputs["b2"], trace=True)


# revision 15
# speedup vs baseline: 1.0979x; 1.0644x over previous
"""Trainium2 Bass kernel for nn_Conv2D3_72026601554290.

Reference computation:
    h = conv7x7_valid(x[4,3,70,70], W1[64,3,7,7]) + b1      -> [4,64,64,64]
    repeat 200x: h = W2 @ h + b2   (1x1 conv, shared weights)

Strategy:
  * The 200 repeated affine steps share one weight matrix, so the tail of
    the network is the affine map h -> W2^200 h + (sum_k W2^k) b2.  We fold
    W2^FUSE (computed in float64 on the host, rounded to f32) into the
    device program: the device runs 200/FUSE GEMM steps.  FUSE=200 folds
    everything into the conv weights (a single fused conv).  Numerics vs
    the f32 reference are ~1e-6 for every FUSE (spectral radius of W2 is
    0.979; all intermediate values stay tiny).
  * Data parallel across 8 NeuronCores: 16384 output positions -> 2048 per
    core (half an image each).  No cross-device communication.
  * Conv is an im2col GEMM with the bias folded in as a constant-1 row:
    K = 3*7*7 + 1 = 148, split into accumulating K=128 + K=20 matmuls.
  * Matmul dtype modes:
      - "f32r"  : TF32 operands (pre-rounded on host), 1 cycle/row.
      - "f32r3" : each operand split hi+lo TF32 terms; 3 matmuls
                  (hi*hi + hi*lo + lo*hi) -> fp32-grade accuracy at 3/8
                  the cost of true fp32 matmul.
      - "f32"   : plain fp32 (2 HW passes, 8 cycles/row).
    f32r matmuls require dst PSUM base partition 0; outputs land in four
    [64, 512] PSUM tiles and the Vector/Scalar engines copy them (with a
    +64 partition shift for the second position group) into the [128,
    1024] output layout.
"""

import numpy as np

import concourse.bacc as bacc
import concourse.tile as tile
import concourse.mybir as mybir
from concourse.bass_utils import run_bass_kernel_spmd

F32 = mybir.dt.float32
F32R = mybir.dt.float32r

N_CORES = 8
N_REPEAT = 200
FUSE = 200  # device runs N_REPEAT//FUSE GEMM steps; 200 -> fully folded conv
MM_DTYPE = "f32r"  # "f32" | "f32r" | "f32r3"
WARMUP_MMS = 0  # f32r matmuls run at fixed rate; PE warmup does not help
POS_PER_CORE = 2048  # 4*64*64 / 8
HALF = POS_PER_CORE // 2  # free-dim size of the [128, 1024] output layout
OH = OW = 64
KH = KW = 7
CIN = 3
CH = 64
K_IM = CIN * KH * KW + 1  # 148: im2col rows + constant-1 bias row
K_LO = K_IM - 128  # 20

_cache = {}


def _build_nc(steps, mode):
    """Build + compile the per-core Bass program (same NEFF for all cores)."""
    nterm = {"f32": 1, "f32r": 1, "f32r3": 2}[mode]  # operand split terms
    mdt = F32 if mode == "f32" else F32R
    warmup = WARMUP_MMS if steps == 0 else 0
    nc = bacc.Bacc("TRN2", target_bir_lowering=False, debug=False,
                   num_devices=N_CORES)

    # packed conv weights: [128, 128] = [ WcT[0:128] | WcT[128:148] in rows 0:20 ]
    wc_ext = [nc.declare_dram_parameter(f"wc{t}", [128, 2 * CH], mdt, isOutput=False)
              for t in range(nterm)]
    im_ext = [nc.declare_dram_parameter(f"im{t}", [K_IM, POS_PER_CORE], mdt,
                                        isOutput=False)
              for t in range(nterm)]
    if steps:
        wl_ext = nc.declare_dram_parameter("wl", [128, CH + 1], F32, isOutput=False)
    o_ext = nc.declare_dram_parameter("o", [128, HALF], F32, isOutput=True)

    with tile.TileContext(nc) as tc:
        with (
            tc.tile_pool(name="const", bufs=1) as cpool,
            tc.tile_pool(name="act", bufs=2) as apool,
            tc.tile_pool(name="psum", bufs=1, space="PSUM") as ppool,
        ):
            wcp = [cpool.tile([128, 2 * CH], mdt, name=f"wcp{t}_sb")
                   for t in range(nterm)]
            imh = [[cpool.tile([128, 512], mdt, name=f"imh{t}c{c}_sb")
                    for c in range(4)] for t in range(nterm)]
            iml = [cpool.tile([K_LO, POS_PER_CORE], mdt, name=f"iml{t}_sb")
                   for t in range(nterm)]
            # no barrier: every conv matmul waits on exactly two DMA queues
            # (its packed-weights load + its im2col chunk load)
            for t in range(nterm):
                nc.sync.dma_start(wcp[t][:], wc_ext[t][:])
                nc.sync.dma_start(iml[t][:], im_ext[t][128:K_IM, :])
                for c in range(4):
                    cs = slice(c * 512, (c + 1) * 512)
                    nc.sync.dma_start(imh[t][c][:], im_ext[t][0:128, cs])
            if steps:
                wl = cpool.tile([128, CH + 1], F32)
                nc.sync.dma_start(wl[:], wl_ext[:])
            # prime the scalar-engine activation table while DMAs stream
            scratch = apool.tile([128, 1], F32, tag="scratch")
            nc.scalar.activation(scratch[:], wcp[0][:, 0:1].bitcast(F32),
                                 mybir.ActivationFunctionType.Identity)

            # ---- conv GEMM: 4 chunks of 512 positions, dst PSUM partitions 0:64
            if nterm == 1:
                pairs = [(0, 0)]
            else:  # hi*hi + hi*lo + lo*hi  (lo*lo term negligible)
                pairs = [(0, 0), (0, 1), (1, 0)]
            ps = [ppool.tile([64, 512], F32, name=f"ps{c}") for c in range(4)]
            for c in range(4):
                cs = slice(c * 512, (c + 1) * 512)
                n = len(pairs)
                for i, (tw, tx) in enumerate(pairs):
                    nc.tensor.matmul(ps[c][:], wcp[tw][:, 0:CH], imh[tx][c][:],
                                     start=(i == 0), stop=False,
                                     tile_position=(0, 0))
                    nc.tensor.matmul(ps[c][:], wcp[tw][0:K_LO, CH:2 * CH],
                                     iml[tx][:, cs],
                                     start=False, stop=(i == n - 1),
                                     tile_position=(0, 0))

            # ---- copies into [128, 1024] layout (+64 partition shift for B),
            # each followed by its own output-store DMA
            h = apool.tile([128, HALF], F32, tag="h")
            nc.vector.tensor_copy(h[0:64, 0:512], ps[0][:])
            if steps == 0:
                nc.sync.dma_start(o_ext[0:64, 0:512], h[0:64, 0:512])
            nc.scalar.copy(h[0:64, 512:1024], ps[1][:])
            if steps == 0:
                nc.scalar.dma_start(o_ext[0:64, 512:1024], h[0:64, 512:1024])
            nc.vector.tensor_copy(h[64:128, 0:512], ps[2][:])
            if steps == 0:
                nc.sync.dma_start(o_ext[64:128, 0:512], h[64:128, 0:512])
            nc.scalar.copy(h[64:128, 512:1024], ps[3][:])
            if steps == 0:
                nc.scalar.dma_start(o_ext[64:128, 512:1024], h[64:128, 512:1024])

            # ---- fused GEMM steps (fp32 exact): h <- P_FUSE @ h + c_FUSE
            for s in range(steps):
                bl = wl[:, CH:CH + 1]
                psA = ppool.tile([128, 512], F32, name="psA", bufs=2)
                psB = ppool.tile([128, 512], F32, name="psB", bufs=2)
                nc.tensor.matmul(psA[0:64, :], wl[0:64, 0:CH], h[0:64, 0:512],
                                 start=True, stop=True, tile_position=(0, 0))
                nc.tensor.matmul(psA[64:128, :], wl[0:64, 0:CH], h[0:64, 512:1024],
                                 start=True, stop=True, tile_position=(0, 64))
                nc.tensor.matmul(psB[0:64, :], wl[64:128, 0:CH], h[64:128, 0:512],
                                 start=True, stop=True, tile_position=(64, 0))
                nc.tensor.matmul(psB[64:128, :], wl[64:128, 0:CH],
                                 h[64:128, 512:1024],
                                 start=True, stop=True, tile_position=(64, 64))
                last = s == steps - 1
                h_new = apool.tile([128, HALF], F32, tag="h")
                nc.vector.tensor_scalar(h_new[:, 0:512], psA[:], bl, None,
                                        mybir.AluOpType.add)
                nc.scalar.activation(h_new[:, 512:1024], psB[:],
                                     mybir.ActivationFunctionType.Identity,
                                     bias=bl)
                if last:
                    nc.sync.dma_start(o_ext[:, 0:512], h_new[:, 0:512])
                    nc.sync.dma_start(o_ext[:, 512:1024], h_new[:, 512:1024])
                h = h_new

    nc.compile()
    return nc


def _fold(W1, b1, W2, b2, fuse):
    """Fold `fuse` affine steps into the conv weights (float64 host math).

    Returns (Wc [64,148] incl bias column, Pk [64,64]|None, ck [64]|None).
    """
    W2d = W2.astype(np.float64)
    W1m = W1.reshape(CH, K_IM - 1).astype(np.float64)

    def affine_pow(k):
        # (P, S) with P = W2^k, S = sum_{j<k} W2^j  via binary doubling
        P = np.eye(CH)
        S = np.zeros((CH, CH))
        base_P = W2d
        base_S = np.eye(CH)
        while k:
            if k & 1:
                S = base_S + base_P @ S
                P = base_P @ P
            base_S = base_S + base_P @ base_S
            base_P = base_P @ base_P
            k >>= 1
        return P, S

    if fuse == N_REPEAT:
        P, S = affine_pow(N_REPEAT)
        Wm = P @ W1m
        bias = P @ b1.astype(np.float64) + S @ b2.astype(np.float64)
    else:
        Wm = W1m
        bias = b1.astype(np.float64)
    Wc = np.concatenate([Wm, bias[:, None]], axis=1)  # [64, 148]
    if fuse == N_REPEAT:
        return Wc, None, None
    P, S = affine_pow(fuse)
    return Wc, P.astype(np.float32), (S @ b2.astype(np.float64)).astype(np.float32)


def _im2col_core(x, core):
    """im2col + constant-1 bias row for this core -> [148, 2048] f64->f32."""
    b = core // 2
    y0 = 32 * (core % 2)
    cols = np.empty((K_IM, POS_PER_CORE), np.float32)
    i = 0
    for c in range(CIN):
        for dy in range(KH):
            for dx in range(KW):
                cols[i] = x[b, c, y0 + dy:y0 + dy + 32, dx:dx + OW].reshape(-1)
                i += 1
    cols[i] = 1.0
    return cols


def _tf32_round(a):
    """Round f32 array to tf32 (10-bit mantissa), round-to-nearest-even."""
    a = np.ascontiguousarray(a, dtype=np.float32)
    u = a.view(np.uint32)
    lsb = (u >> 13) & 1
    out = ((u + 0x0FFF + lsb) & 0xFFFFE000).astype(np.uint32)
    return out.view(np.float32)


def _split_terms(a, mode):
    """Operand splitting per matmul dtype mode -> list of arrays."""
    if mode == "f32":
        return [np.ascontiguousarray(a, dtype=np.float32)]
    hi = _tf32_round(a)
    if mode == "f32r":
        return [hi]
    lo = _tf32_round(np.asarray(a, np.float32) - hi)
    return [hi, lo]


def _run(x, W1, b1, W2, b2, trace=False):
    x = np.asarray(x, dtype=np.float32)
    W1 = np.asarray(W1, dtype=np.float32)
    b1 = np.asarray(b1, dtype=np.float32)
    W2 = np.asarray(W2, dtype=np.float32)
    b2 = np.asarray(b2, dtype=np.float32)

    steps = 0 if FUSE == N_REPEAT else N_REPEAT // FUSE
    if steps:
        assert steps * FUSE == N_REPEAT

    key = (steps, MM_DTYPE, WARMUP_MMS)
    if _cache.get("key") != key:
        _cache["nc"] = _build_nc(steps, MM_DTYPE)
        _cache["key"] = key
    nc = _cache["nc"]

    nterm = {"f32": 1, "f32r": 1, "f32r3": 2}[MM_DTYPE]  # operand terms

    Wc, Pk, ck = _fold(W1, b1, W2, b2, FUSE)
    WcT = np.ascontiguousarray(Wc.T)  # [148, 64] lhsT layout
    w_terms = _split_terms(WcT, MM_DTYPE)

    shared = {}
    for t, wt in enumerate(w_terms):
        pack = np.zeros((128, 2 * CH), np.float32)
        pack[:, 0:CH] = wt[0:128]
        pack[0:K_LO, CH:2 * CH] = wt[128:K_IM]
        shared[f"wc{t}"] = pack
    if steps:
        PkT = np.ascontiguousarray(Pk.T)
        wl = np.concatenate([PkT, PkT], axis=0).astype(np.float32)
        bl = np.concatenate([ck, ck])[:, None].astype(np.float32)
        shared["wl"] = np.concatenate([wl, bl], axis=1)

    in_maps = []
    for core in range(N_CORES):
        cols = _im2col_core(x, core)
        x_terms = _split_terms(cols, MM_DTYPE)
        m = dict(shared)
        for t, arr in enumerate(x_terms):
            m[f"im{t}"] = arr
        in_maps.append(m)

    res = run_bass_kernel_spmd(nc, in_maps, list(range(N_CORES)), trace=trace)

    out = np.empty((4, CH, OH, OW), np.float32)
    for core in range(N_CORES):
        o = res.results[core]["o"].copy()
        if steps % 2 == 1:
            # undo the per-step quarter-block swap (Q2 <-> Q3)
            tmp = o[0:64, 512:1024].copy()
            o[0:64, 512:1024] = o[64:128, 0:512]
            o[64:128, 0:512] = tmp
        b = core // 2
        y0 = 32 * (core % 2)
        # group A = local positions 0..1023 (16 rows), group B = 1024..2047
        out[b, :, y0:y0 + 16, :] = o[0:64].reshape(CH, 16, OW)
        out[b, :, y0 + 16:y0 + 32, :] = o[64:128].reshape(CH, 16, OW)
    return out, res


def kernel(**inputs):
    out, _ = _run(inputs["x"], inputs["W1"], inputs["b1"],
                  inputs["W2"], inputs["b2"], trace=False)
    return out


def kernel_traced(**inputs):
    """Like kernel() but with NTFF hardware profiling; returns (out, res)."""
    import sys
    import types
    if "antenv.axon_hooks" not in sys.modules:
        from trn_agent_boot.trn_boot import _ntff_profile_via_ctypes
        hook = _ntff_profile_via_ctypes("/opt/axon/libaxon_pjrt.so")
        mod = types.ModuleType("antenv.axon_hooks")
        mod.get_axon_ntff_profile_hook = lambda: hook
        mod.set_axon_ntff_profile_hook = lambda h: None
        sys.modules["antenv.axon_hooks"] = mod
    return _run(inputs["x"], inputs["W1"], inputs["b1"],
                inputs["W2"], inputs["b2"], trace=True)


# revision 16
# speedup vs baseline: 1.1449x; 1.0428x over previous
"""Trainium2 Bass kernel for nn_Conv2D3_72026601554290.

Reference computation:
    h = conv7x7_valid(x[4,3,70,70], W1[64,3,7,7]) + b1      -> [4,64,64,64]
    repeat 200x: h = W2 @ h + b2   (1x1 conv, shared weights)

Strategy:
  * The 200 repeated affine steps share one weight matrix, so the tail of
    the network is the affine map h -> W2^200 h + (sum_k W2^k) b2.  We fold
    W2^FUSE (computed in float64 on the host, rounded to f32) into the
    device program: the device runs 200/FUSE GEMM steps.  FUSE=200 folds
    everything into the conv weights (a single fused conv).  Numerics vs
    the f32 reference are ~1e-6 for every FUSE (spectral radius of W2 is
    0.979; all intermediate values stay tiny).
  * Data parallel across 8 NeuronCores: 16384 output positions -> 2048 per
    core (half an image each).  No cross-device communication.
  * Conv is an im2col GEMM with the bias folded in as a constant-1 row:
    K = 3*7*7 + 1 = 148, split into accumulating K=128 + K=20 matmuls.
  * Matmul dtype modes:
      - "f32r"  : TF32 operands (pre-rounded on host), 1 cycle/row.
      - "f32r3" : each operand split hi+lo TF32 terms; 3 matmuls
                  (hi*hi + hi*lo + lo*hi) -> fp32-grade accuracy at 3/8
                  the cost of true fp32 matmul.
      - "f32"   : plain fp32 (2 HW passes, 8 cycles/row).
    f32r matmuls require dst PSUM base partition 0; outputs land in four
    [64, 512] PSUM tiles and the Vector/Scalar engines copy them (with a
    +64 partition shift for the second position group) into the [128,
    1024] output layout.
"""

import numpy as np

import concourse.bacc as bacc
import concourse.tile as tile
import concourse.mybir as mybir
from concourse.bass_utils import run_bass_kernel_spmd

F32 = mybir.dt.float32
F32R = mybir.dt.float32r

N_CORES = 8
N_REPEAT = 200
FUSE = 200  # device runs N_REPEAT//FUSE GEMM steps; 200 -> fully folded conv
MM_DTYPE = "f32r"  # "f32" | "f32r" | "f32r3"
WARMUP_MMS = 0  # f32r matmuls run at fixed rate; PE warmup does not help
POS_PER_CORE = 2048  # 4*64*64 / 8
HALF = POS_PER_CORE // 2  # free-dim size of the [128, 1024] output layout
OH = OW = 64
KH = KW = 7
CIN = 3
CH = 64
K_IM = CIN * KH * KW + 1  # 148: im2col rows + constant-1 bias row
K_LO = K_IM - 128  # 20

_cache = {}


def _build_nc(steps, mode):
    """Build + compile the per-core Bass program (same NEFF for all cores)."""
    nterm = {"f32": 1, "f32r": 1, "f32r3": 2}[mode]  # operand split terms
    mdt = F32 if mode == "f32" else F32R
    warmup = WARMUP_MMS if steps == 0 else 0
    nc = bacc.Bacc("TRN2", target_bir_lowering=False, debug=False,
                   num_devices=N_CORES)

    # packed conv weights: [128, 128] = [ WcT[0:128] | WcT[128:148] in rows 0:20 ]
    wc_ext = [nc.declare_dram_parameter(f"wc{t}", [128, 2 * CH], mdt, isOutput=False)
              for t in range(nterm)]
    # im2col stored chunk-major: [512, 512] = 4 stacked [128, 512] chunks,
    # so each chunk DMA reads one fully contiguous 256KB block
    im_ext = [nc.declare_dram_parameter(f"im{t}", [512, 512], mdt, isOutput=False)
              for t in range(nterm)]
    iml_ext = [nc.declare_dram_parameter(f"iml{t}", [K_LO, POS_PER_CORE], mdt,
                                         isOutput=False)
               for t in range(nterm)]
    if steps:
        wl_ext = nc.declare_dram_parameter("wl", [128, CH + 1], F32, isOutput=False)
    o_ext = nc.declare_dram_parameter("o", [128, HALF], F32, isOutput=True)

    with tile.TileContext(nc) as tc:
        with (
            tc.tile_pool(name="const", bufs=1) as cpool,
            tc.tile_pool(name="act", bufs=2) as apool,
            tc.tile_pool(name="psum", bufs=1, space="PSUM") as ppool,
        ):
            wcp = [cpool.tile([128, 2 * CH], mdt, name=f"wcp{t}_sb")
                   for t in range(nterm)]
            imh = [[cpool.tile([128, 512], mdt, name=f"imh{t}c{c}_sb")
                    for c in range(4)] for t in range(nterm)]
            iml = [cpool.tile([K_LO, POS_PER_CORE], mdt, name=f"iml{t}_sb")
                   for t in range(nterm)]
            # no barrier: every conv matmul waits on exactly two DMA queues
            # (its packed-weights load + its im2col chunk load)
            for t in range(nterm):
                nc.sync.dma_start(wcp[t][:], wc_ext[t][:])
                nc.sync.dma_start(imh[t][0][:], im_ext[t][0:128, :])
                nc.sync.dma_start(iml[t][:], iml_ext[t][:])
                for c in range(1, 4):
                    nc.sync.dma_start(imh[t][c][:],
                                      im_ext[t][c * 128:(c + 1) * 128, :])
            if steps:
                wl = cpool.tile([128, CH + 1], F32)
                nc.sync.dma_start(wl[:], wl_ext[:])
            # prime the scalar-engine activation table (no data dependency)
            scratch = apool.tile([128, 1], F32, tag="scratch")
            nc.vector.memset(scratch[:], 0.0)
            nc.scalar.activation(scratch[:], scratch[:],
                                 mybir.ActivationFunctionType.Identity)

            # ---- conv GEMM: 4 chunks of 512 positions, dst PSUM partitions 0:64
            if nterm == 1:
                pairs = [(0, 0)]
            else:  # hi*hi + hi*lo + lo*hi  (lo*lo term negligible)
                pairs = [(0, 0), (0, 1), (1, 0)]
            ps = [ppool.tile([64, 512], F32, name=f"ps{c}") for c in range(4)]
            for c in range(4):
                cs = slice(c * 512, (c + 1) * 512)
                n = len(pairs)
                for i, (tw, tx) in enumerate(pairs):
                    nc.tensor.matmul(ps[c][:], wcp[tw][:, 0:CH], imh[tx][c][:],
                                     start=(i == 0), stop=False,
                                     tile_position=(0, 0))
                    nc.tensor.matmul(ps[c][:], wcp[tw][0:K_LO, CH:2 * CH],
                                     iml[tx][:, cs],
                                     start=False, stop=(i == n - 1),
                                     tile_position=(0, 0))

            # ---- copies into [128, 1024] layout (+64 partition shift for B),
            # each followed by its own output-store DMA
            h = apool.tile([128, HALF], F32, tag="h")
            nc.vector.tensor_copy(h[0:64, 0:512], ps[0][:])
            if steps == 0:
                nc.sync.dma_start(o_ext[0:64, 0:512], h[0:64, 0:512])
            nc.scalar.copy(h[0:64, 512:1024], ps[1][:])
            if steps == 0:
                nc.scalar.dma_start(o_ext[0:64, 512:1024], h[0:64, 512:1024])
            nc.vector.tensor_copy(h[64:128, 0:512], ps[2][:])
            if steps == 0:
                nc.sync.dma_start(o_ext[64:128, 0:512], h[64:128, 0:512])
            nc.scalar.copy(h[64:128, 512:1024], ps[3][:])
            if steps == 0:
                nc.scalar.dma_start(o_ext[64:128, 512:1024], h[64:128, 512:1024])

            # ---- fused GEMM steps (fp32 exact): h <- P_FUSE @ h + c_FUSE
            for s in range(steps):
                bl = wl[:, CH:CH + 1]
                psA = ppool.tile([128, 512], F32, name="psA", bufs=2)
                psB = ppool.tile([128, 512], F32, name="psB", bufs=2)
                nc.tensor.matmul(psA[0:64, :], wl[0:64, 0:CH], h[0:64, 0:512],
                                 start=True, stop=True, tile_position=(0, 0))
                nc.tensor.matmul(psA[64:128, :], wl[0:64, 0:CH], h[0:64, 512:1024],
                                 start=True, stop=True, tile_position=(0, 64))
                nc.tensor.matmul(psB[0:64, :], wl[64:128, 0:CH], h[64:128, 0:512],
                                 start=True, stop=True, tile_position=(64, 0))
                nc.tensor.matmul(psB[64:128, :], wl[64:128, 0:CH],
                                 h[64:128, 512:1024],
                                 start=True, stop=True, tile_position=(64, 64))
                last = s == steps - 1
                h_new = apool.tile([128, HALF], F32, tag="h")
                nc.vector.tensor_scalar(h_new[:, 0:512], psA[:], bl, None,
                                        mybir.AluOpType.add)
                nc.scalar.activation(h_new[:, 512:1024], psB[:],
                                     mybir.ActivationFunctionType.Identity,
                                     bias=bl)
                if last:
                    nc.sync.dma_start(o_ext[:, 0:512], h_new[:, 0:512])
                    nc.sync.dma_start(o_ext[:, 512:1024], h_new[:, 512:1024])
                h = h_new

    nc.compile()
    return nc


def _fold(W1, b1, W2, b2, fuse):
    """Fold `fuse` affine steps into the conv weights (float64 host math).

    Returns (Wc [64,148] incl bias column, Pk [64,64]|None, ck [64]|None).
    """
    W2d = W2.astype(np.float64)
    W1m = W1.reshape(CH, K_IM - 1).astype(np.float64)

    def affine_pow(k):
        # (P, S) with P = W2^k, S = sum_{j<k} W2^j  via binary doubling
        P = np.eye(CH)
        S = np.zeros((CH, CH))
        base_P = W2d
        base_S = np.eye(CH)
        while k:
            if k & 1:
                S = base_S + base_P @ S
                P = base_P @ P
            base_S = base_S + base_P @ base_S
            base_P = base_P @ base_P
            k >>= 1
        return P, S

    if fuse == N_REPEAT:
        P, S = affine_pow(N_REPEAT)
        Wm = P @ W1m
        bias = P @ b1.astype(np.float64) + S @ b2.astype(np.float64)
    else:
        Wm = W1m
        bias = b1.astype(np.float64)
    Wc = np.concatenate([Wm, bias[:, None]], axis=1)  # [64, 148]
    if fuse == N_REPEAT:
        return Wc, None, None
    P, S = affine_pow(fuse)
    return Wc, P.astype(np.float32), (S @ b2.astype(np.float64)).astype(np.float32)


def _im2col_core(x, core):
    """im2col + constant-1 bias row for this core -> [148, 2048] f64->f32."""
    b = core // 2
    y0 = 32 * (core % 2)
    cols = np.empty((K_IM, POS_PER_CORE), np.float32)
    i = 0
    for c in range(CIN):
        for dy in range(KH):
            for dx in range(KW):
                cols[i] = x[b, c, y0 + dy:y0 + dy + 32, dx:dx + OW].reshape(-1)
                i += 1
    cols[i] = 1.0
    return cols


def _tf32_round(a):
    """Round f32 array to tf32 (10-bit mantissa), round-to-nearest-even."""
    a = np.ascontiguousarray(a, dtype=np.float32)
    u = a.view(np.uint32)
    lsb = (u >> 13) & 1
    out = ((u + 0x0FFF + lsb) & 0xFFFFE000).astype(np.uint32)
    return out.view(np.float32)


def _split_terms(a, mode):
    """Operand splitting per matmul dtype mode -> list of arrays."""
    if mode == "f32":
        return [np.ascontiguousarray(a, dtype=np.float32)]
    hi = _tf32_round(a)
    if mode == "f32r":
        return [hi]
    lo = _tf32_round(np.asarray(a, np.float32) - hi)
    return [hi, lo]


def _run(x, W1, b1, W2, b2, trace=False):
    x = np.asarray(x, dtype=np.float32)
    W1 = np.asarray(W1, dtype=np.float32)
    b1 = np.asarray(b1, dtype=np.float32)
    W2 = np.asarray(W2, dtype=np.float32)
    b2 = np.asarray(b2, dtype=np.float32)

    steps = 0 if FUSE == N_REPEAT else N_REPEAT // FUSE
    if steps:
        assert steps * FUSE == N_REPEAT

    key = (steps, MM_DTYPE, WARMUP_MMS)
    if _cache.get("key") != key:
        _cache["nc"] = _build_nc(steps, MM_DTYPE)
        _cache["key"] = key
    nc = _cache["nc"]

    nterm = {"f32": 1, "f32r": 1, "f32r3": 2}[MM_DTYPE]  # operand terms

    Wc, Pk, ck = _fold(W1, b1, W2, b2, FUSE)
    WcT = np.ascontiguousarray(Wc.T)  # [148, 64] lhsT layout
    w_terms = _split_terms(WcT, MM_DTYPE)

    shared = {}
    for t, wt in enumerate(w_terms):
        pack = np.zeros((128, 2 * CH), np.float32)
        pack[:, 0:CH] = wt[0:128]
        pack[0:K_LO, CH:2 * CH] = wt[128:K_IM]
        shared[f"wc{t}"] = pack
    if steps:
        PkT = np.ascontiguousarray(Pk.T)
        wl = np.concatenate([PkT, PkT], axis=0).astype(np.float32)
        bl = np.concatenate([ck, ck])[:, None].astype(np.float32)
        shared["wl"] = np.concatenate([wl, bl], axis=1)

    in_maps = []
    for core in range(N_CORES):
        cols = _im2col_core(x, core)
        x_terms = _split_terms(cols, MM_DTYPE)
        m = dict(shared)
        for t, arr in enumerate(x_terms):
            # chunk-major layout: [512, 512] stacked [128, 512] chunks
            m[f"im{t}"] = np.ascontiguousarray(
                np.concatenate([arr[0:128, c * 512:(c + 1) * 512]
                                for c in range(4)], axis=0))
            m[f"iml{t}"] = np.ascontiguousarray(arr[128:K_IM, :])
        in_maps.append(m)

    res = run_bass_kernel_spmd(nc, in_maps, list(range(N_CORES)), trace=trace)

    out = np.empty((4, CH, OH, OW), np.float32)
    for core in range(N_CORES):
        o = res.results[core]["o"].copy()
        if steps % 2 == 1:
            # undo the per-step quarter-block swap (Q2 <-> Q3)
            tmp = o[0:64, 512:1024].copy()
            o[0:64, 512:1024] = o[64:128, 0:512]
            o[64:128, 0:512] = tmp
        b = core // 2
        y0 = 32 * (core % 2)
        # group A = local positions 0..1023 (16 rows), group B = 1024..2047
        out[b, :, y0:y0 + 16, :] = o[0:64].reshape(CH, 16, OW)
        out[b, :, y0 + 16:y0 + 32, :] = o[64:128].reshape(CH, 16, OW)
    return out, res


def kernel(**inputs):
    out, _ = _run(inputs["x"], inputs["W1"], inputs["b1"],
                  inputs["W2"], inputs["b2"], trace=False)
    return out


def kernel_traced(**inputs):
    """Like kernel() but with NTFF hardware profiling; returns (out, res)."""
    import sys
    import types
    if "antenv.axon_hooks" not in sys.modules:
        from trn_agent_boot.trn_boot import _ntff_profile_via_ctypes
        hook = _ntff_profile_via_ctypes("/opt/axon/libaxon_pjrt.so")
        mod = types.ModuleType("antenv.axon_hooks")
        mod.get_axon_ntff_profile_hook = lambda: hook
        mod.set_axon_ntff_profile_hook = lambda h: None
        sys.modules["antenv.axon_hooks"] = mod
    return _run(inputs["x"], inputs["W1"], inputs["b1"],
                inputs["W2"], inputs["b2"], trace=True)


# revision 17
# speedup vs baseline: 1.1802x; 1.0308x over previous
"""Trainium2 Bass kernel for nn_Conv2D3_72026601554290.

Reference computation:
    h = conv7x7_valid(x[4,3,70,70], W1[64,3,7,7]) + b1      -> [4,64,64,64]
    repeat 200x: h = W2 @ h + b2   (1x1 conv, shared weights)

Strategy:
  * The 200 repeated affine steps share one weight matrix, so the tail of
    the network is the affine map h -> W2^200 h + (sum_k W2^k) b2.  We fold
    W2^FUSE (computed in float64 on the host, rounded to f32) into the
    device program: the device runs 200/FUSE GEMM steps.  FUSE=200 folds
    everything into the conv weights (a single fused conv).  Numerics vs
    the f32 reference are ~1e-6 for every FUSE (spectral radius of W2 is
    0.979; all intermediate values stay tiny).
  * Data parallel across 8 NeuronCores: 16384 output positions -> 2048 per
    core (half an image each).  No cross-device communication.
  * Conv is an im2col GEMM with the bias folded in as a constant-1 row:
    K = 3*7*7 + 1 = 148, split into accumulating K=128 + K=20 matmuls.
  * Matmul dtype modes:
      - "f32r"  : TF32 operands (pre-rounded on host), 1 cycle/row.
      - "f32r3" : each operand split hi+lo TF32 terms; 3 matmuls
                  (hi*hi + hi*lo + lo*hi) -> fp32-grade accuracy at 3/8
                  the cost of true fp32 matmul.
      - "f32"   : plain fp32 (2 HW passes, 8 cycles/row).
    f32r matmuls require dst PSUM base partition 0; outputs land in four
    [64, 512] PSUM tiles and the Vector/Scalar engines copy them (with a
    +64 partition shift for the second position group) into the [128,
    1024] output layout.
"""

import numpy as np

import concourse.bacc as bacc
import concourse.tile as tile
import concourse.mybir as mybir
from concourse.bass_utils import run_bass_kernel_spmd

F32 = mybir.dt.float32
F32R = mybir.dt.float32r

N_CORES = 8
N_REPEAT = 200
FUSE = 200  # device runs N_REPEAT//FUSE GEMM steps; 200 -> fully folded conv
MM_DTYPE = "f32r"  # "f32" | "f32r" | "f32r3"
WARMUP_MMS = 0  # f32r matmuls run at fixed rate; PE warmup does not help
POS_PER_CORE = 2048  # 4*64*64 / 8
HALF = POS_PER_CORE // 2  # free-dim size of the [128, 1024] output layout
OH = OW = 64
KH = KW = 7
CIN = 3
CH = 64
K_IM = CIN * KH * KW + 1  # 148: im2col rows + constant-1 bias row
K_LO = K_IM - 128  # 20

_cache = {}


def _build_nc(steps, mode):
    """Build + compile the per-core Bass program (same NEFF for all cores)."""
    nterm = {"f32": 1, "f32r": 1, "f32r3": 2}[mode]  # operand split terms
    mdt = F32 if mode == "f32" else F32R
    warmup = WARMUP_MMS if steps == 0 else 0
    nc = bacc.Bacc("TRN2", target_bir_lowering=False, debug=False,
                   num_devices=N_CORES)

    # packed conv weights: [128, 128] = [ WcT[0:128] | WcT[128:148] in rows 0:20 ]
    wc_ext = [nc.declare_dram_parameter(f"wc{t}", [128, 2 * CH], mdt, isOutput=False)
              for t in range(nterm)]
    # im2col stored chunk-major: [512, 512] = 4 stacked [128, 512] chunks,
    # so each chunk DMA reads one fully contiguous 256KB block
    im_ext = [nc.declare_dram_parameter(f"im{t}", [512, 512], mdt, isOutput=False)
              for t in range(nterm)]
    # K=20 tail rows, one 32-row block per chunk (rows 32c:32c+20), padded
    # with zeros -> full-partition contiguous transfer + per-chunk row tiles
    iml_ext = [nc.declare_dram_parameter(f"iml{t}", [128, 512], mdt,
                                         isOutput=False)
               for t in range(nterm)]
    if steps:
        wl_ext = nc.declare_dram_parameter("wl", [128, CH + 1], F32, isOutput=False)
    o_ext = nc.declare_dram_parameter("o", [128, HALF], F32, isOutput=True)

    with tile.TileContext(nc) as tc:
        with (
            tc.tile_pool(name="const", bufs=1) as cpool,
            tc.tile_pool(name="act", bufs=2) as apool,
            tc.tile_pool(name="psum", bufs=1, space="PSUM") as ppool,
        ):
            wcp = [cpool.tile([128, 2 * CH], mdt, name=f"wcp{t}_sb")
                   for t in range(nterm)]
            imh = [[cpool.tile([128, 512], mdt, name=f"imh{t}c{c}_sb")
                    for c in range(4)] for t in range(nterm)]
            iml = [cpool.tile([128, 512], mdt, name=f"iml{t}_sb")
                   for t in range(nterm)]
            # no barrier: every conv matmul waits on exactly two DMA queues
            # (its packed-weights load + its im2col chunk load)
            # input loads split across the two HWDGE trigger engines
            # (sync + scalar) so the two queues stream concurrently
            for t in range(nterm):
                nc.sync.dma_start(wcp[t][:], wc_ext[t][:])
                nc.scalar.dma_start(iml[t][:], iml_ext[t][:])
                nc.sync.dma_start(imh[t][0][:], im_ext[t][0:128, :])
                nc.scalar.dma_start(imh[t][1][:], im_ext[t][128:256, :])
                nc.sync.dma_start(imh[t][2][:], im_ext[t][256:384, :])
                nc.scalar.dma_start(imh[t][3][:], im_ext[t][384:512, :])
            if steps:
                wl = cpool.tile([128, CH + 1], F32)
                nc.sync.dma_start(wl[:], wl_ext[:])
            # prime the scalar-engine activation table (emitted after the
            # scalar-issued DMA triggers; no data dependency)
            scratch = apool.tile([128, 1], F32, tag="scratch")
            nc.vector.memset(scratch[:], 0.0)
            nc.scalar.activation(scratch[:], scratch[:],
                                 mybir.ActivationFunctionType.Identity)

            # ---- conv GEMM: 4 chunks of 512 positions, dst PSUM partitions 0:64
            if nterm == 1:
                pairs = [(0, 0)]
            else:  # hi*hi + hi*lo + lo*hi  (lo*lo term negligible)
                pairs = [(0, 0), (0, 1), (1, 0)]
            ps = [ppool.tile([64, 512], F32, name=f"ps{c}") for c in range(4)]
            for c in range(4):
                cs = slice(c * 512, (c + 1) * 512)
                n = len(pairs)
                for i, (tw, tx) in enumerate(pairs):
                    nc.tensor.matmul(ps[c][:], wcp[tw][:, 0:CH], imh[tx][c][:],
                                     start=(i == 0), stop=False,
                                     tile_position=(0, 0))
                    r0 = 32 * c
                    nc.tensor.matmul(ps[c][:], wcp[tw][r0:r0 + 32, CH:2 * CH],
                                     iml[tx][r0:r0 + 32, :],
                                     start=False, stop=(i == n - 1),
                                     tile_position=(r0, 0))

            # ---- copies into [128, 1024] layout (+64 partition shift for B),
            # each followed by its own output-store DMA
            h = apool.tile([128, HALF], F32, tag="h")
            nc.vector.tensor_copy(h[0:64, 0:512], ps[0][:])
            if steps == 0:
                nc.sync.dma_start(o_ext[0:64, 0:512], h[0:64, 0:512])
            nc.scalar.copy(h[0:64, 512:1024], ps[1][:])
            if steps == 0:
                nc.scalar.dma_start(o_ext[0:64, 512:1024], h[0:64, 512:1024])
            nc.vector.tensor_copy(h[64:128, 0:512], ps[2][:])
            if steps == 0:
                nc.sync.dma_start(o_ext[64:128, 0:512], h[64:128, 0:512])
            nc.scalar.copy(h[64:128, 512:1024], ps[3][:])
            if steps == 0:
                nc.scalar.dma_start(o_ext[64:128, 512:1024], h[64:128, 512:1024])

            # ---- fused GEMM steps (fp32 exact): h <- P_FUSE @ h + c_FUSE
            for s in range(steps):
                bl = wl[:, CH:CH + 1]
                psA = ppool.tile([128, 512], F32, name="psA", bufs=2)
                psB = ppool.tile([128, 512], F32, name="psB", bufs=2)
                nc.tensor.matmul(psA[0:64, :], wl[0:64, 0:CH], h[0:64, 0:512],
                                 start=True, stop=True, tile_position=(0, 0))
                nc.tensor.matmul(psA[64:128, :], wl[0:64, 0:CH], h[0:64, 512:1024],
                                 start=True, stop=True, tile_position=(0, 64))
                nc.tensor.matmul(psB[0:64, :], wl[64:128, 0:CH], h[64:128, 0:512],
                                 start=True, stop=True, tile_position=(64, 0))
                nc.tensor.matmul(psB[64:128, :], wl[64:128, 0:CH],
                                 h[64:128, 512:1024],
                                 start=True, stop=True, tile_position=(64, 64))
                last = s == steps - 1
                h_new = apool.tile([128, HALF], F32, tag="h")
                nc.vector.tensor_scalar(h_new[:, 0:512], psA[:], bl, None,
                                        mybir.AluOpType.add)
                nc.scalar.activation(h_new[:, 512:1024], psB[:],
                                     mybir.ActivationFunctionType.Identity,
                                     bias=bl)
                if last:
                    nc.sync.dma_start(o_ext[:, 0:512], h_new[:, 0:512])
                    nc.sync.dma_start(o_ext[:, 512:1024], h_new[:, 512:1024])
                h = h_new

    nc.compile()
    return nc


def _fold(W1, b1, W2, b2, fuse):
    """Fold `fuse` affine steps into the conv weights (float64 host math).

    Returns (Wc [64,148] incl bias column, Pk [64,64]|None, ck [64]|None).
    """
    W2d = W2.astype(np.float64)
    W1m = W1.reshape(CH, K_IM - 1).astype(np.float64)

    def affine_pow(k):
        # (P, S) with P = W2^k, S = sum_{j<k} W2^j  via binary doubling
        P = np.eye(CH)
        S = np.zeros((CH, CH))
        base_P = W2d
        base_S = np.eye(CH)
        while k:
            if k & 1:
                S = base_S + base_P @ S
                P = base_P @ P
            base_S = base_S + base_P @ base_S
            base_P = base_P @ base_P
            k >>= 1
        return P, S

    if fuse == N_REPEAT:
        P, S = affine_pow(N_REPEAT)
        Wm = P @ W1m
        bias = P @ b1.astype(np.float64) + S @ b2.astype(np.float64)
    else:
        Wm = W1m
        bias = b1.astype(np.float64)
    Wc = np.concatenate([Wm, bias[:, None]], axis=1)  # [64, 148]
    if fuse == N_REPEAT:
        return Wc, None, None
    P, S = affine_pow(fuse)
    return Wc, P.astype(np.float32), (S @ b2.astype(np.float64)).astype(np.float32)


def _im2col_core(x, core):
    """im2col + constant-1 bias row for this core -> [148, 2048] f64->f32."""
    b = core // 2
    y0 = 32 * (core % 2)
    cols = np.empty((K_IM, POS_PER_CORE), np.float32)
    i = 0
    for c in range(CIN):
        for dy in range(KH):
            for dx in range(KW):
                cols[i] = x[b, c, y0 + dy:y0 + dy + 32, dx:dx + OW].reshape(-1)
                i += 1
    cols[i] = 1.0
    return cols


def _tf32_round(a):
    """Round f32 array to tf32 (10-bit mantissa), round-to-nearest-even."""
    a = np.ascontiguousarray(a, dtype=np.float32)
    u = a.view(np.uint32)
    lsb = (u >> 13) & 1
    out = ((u + 0x0FFF + lsb) & 0xFFFFE000).astype(np.uint32)
    return out.view(np.float32)


def _split_terms(a, mode):
    """Operand splitting per matmul dtype mode -> list of arrays."""
    if mode == "f32":
        return [np.ascontiguousarray(a, dtype=np.float32)]
    hi = _tf32_round(a)
    if mode == "f32r":
        return [hi]
    lo = _tf32_round(np.asarray(a, np.float32) - hi)
    return [hi, lo]


def _run(x, W1, b1, W2, b2, trace=False):
    x = np.asarray(x, dtype=np.float32)
    W1 = np.asarray(W1, dtype=np.float32)
    b1 = np.asarray(b1, dtype=np.float32)
    W2 = np.asarray(W2, dtype=np.float32)
    b2 = np.asarray(b2, dtype=np.float32)

    steps = 0 if FUSE == N_REPEAT else N_REPEAT // FUSE
    if steps:
        assert steps * FUSE == N_REPEAT

    key = (steps, MM_DTYPE, WARMUP_MMS)
    if _cache.get("key") != key:
        _cache["nc"] = _build_nc(steps, MM_DTYPE)
        _cache["key"] = key
    nc = _cache["nc"]

    nterm = {"f32": 1, "f32r": 1, "f32r3": 2}[MM_DTYPE]  # operand terms

    Wc, Pk, ck = _fold(W1, b1, W2, b2, FUSE)
    WcT = np.ascontiguousarray(Wc.T)  # [148, 64] lhsT layout
    w_terms = _split_terms(WcT, MM_DTYPE)

    shared = {}
    for t, wt in enumerate(w_terms):
        pack = np.zeros((128, 2 * CH), np.float32)
        pack[:, 0:CH] = wt[0:128]
        for c in range(4):  # K=20 tail weights replicated per 32-row block
            pack[32 * c:32 * c + K_LO, CH:2 * CH] = wt[128:K_IM]
        shared[f"wc{t}"] = pack
    if steps:
        PkT = np.ascontiguousarray(Pk.T)
        wl = np.concatenate([PkT, PkT], axis=0).astype(np.float32)
        bl = np.concatenate([ck, ck])[:, None].astype(np.float32)
        shared["wl"] = np.concatenate([wl, bl], axis=1)

    in_maps = []
    for core in range(N_CORES):
        cols = _im2col_core(x, core)
        x_terms = _split_terms(cols, MM_DTYPE)
        m = dict(shared)
        for t, arr in enumerate(x_terms):
            # chunk-major layout: [512, 512] stacked [128, 512] chunks
            m[f"im{t}"] = np.ascontiguousarray(
                np.concatenate([arr[0:128, c * 512:(c + 1) * 512]
                                for c in range(4)], axis=0))
            tail = np.zeros((128, 512), np.float32)
            for c in range(4):
                tail[32 * c:32 * c + K_LO, :] = arr[128:K_IM,
                                                    c * 512:(c + 1) * 512]
            m[f"iml{t}"] = tail
        in_maps.append(m)

    res = run_bass_kernel_spmd(nc, in_maps, list(range(N_CORES)), trace=trace)

    out = np.empty((4, CH, OH, OW), np.float32)
    for core in range(N_CORES):
        o = res.results[core]["o"].copy()
        if steps % 2 == 1:
            # undo the per-step quarter-block swap (Q2 <-> Q3)
            tmp = o[0:64, 512:1024].copy()
            o[0:64, 512:1024] = o[64:128, 0:512]
            o[64:128, 0:512] = tmp
        b = core // 2
        y0 = 32 * (core % 2)
        # group A = local positions 0..1023 (16 rows), group B = 1024..2047
        out[b, :, y0:y0 + 16, :] = o[0:64].reshape(CH, 16, OW)
        out[b, :, y0 + 16:y0 + 32, :] = o[64:128].reshape(CH, 16, OW)
    return out, res


def kernel(**inputs):
    out, _ = _run(inputs["x"], inputs["W1"], inputs["b1"],
                  inputs["W2"], inputs["b2"], trace=False)
    return out


def kernel_traced(**inputs):
    """Like kernel() but with NTFF hardware profiling; returns (out, res)."""
    import sys
    import types
    if "antenv.axon_hooks" not in sys.modules:
        from trn_agent_boot.trn_boot import _ntff_profile_via_ctypes
        hook = _ntff_profile_via_ctypes("/opt/axon/libaxon_pjrt.so")
        mod = types.ModuleType("antenv.axon_hooks")
        mod.get_axon_ntff_profile_hook = lambda: hook
        mod.set_axon_ntff_profile_hook = lambda h: None
        sys.modules["antenv.axon_hooks"] = mod
    return _run(inputs["x"], inputs["W1"], inputs["b1"],
                inputs["W2"], inputs["b2"], trace=True)
